# revision 32
# baseline (speedup 1.0000x reference)
"""Chamfer distance (B=8, N=M=4096, D=3) on 8 Trainium2 NeuronCores.

Strategy: data-parallel over batch -- core b computes batch element b.

The squared-distance matrix is produced NEGATED on the tensor engine via
homogeneous coordinates in bf16 with 3-way hi/mid/lo splitting:
  x ~ x0+x1+x2, 2y ~ c0+c1+c2 (each bf16), products kept for i+j<=2.
With norm rows split the same way the augmented contraction has K=24
rows; sum_k a_k b_k = 2<x,y> - |x|^2 - |y|^2 = -d2[n,m] (+ ~1e-6 err).
The augmented matrices are built host-side in numpy (bf16 bit twiddling)
and DMA'd in, so the device prologue is just two small loads.

Main loop, per (g, i) tile [128 x 2048] in PSUM (2 slots, 4 banks each):
  - 4 matmuls (bf16, 1 cycle/row) write -d2.
  - Eviction to fp16 SBUF is column-split across two engines:
      Act copies cols [0:1536), Pool copies [1536:2048).
  - dist1 side: one DVE tensor_scalar (4x mode) with max-accum ->
      per-tile row-max column in `colmax`.
  - dist2 side: running column max `runs[g]`, column-split:
      DVE tensor_max on [0:1088), Pool tensor_max on [1088:2048).
Engine loads are balanced: Act ~ Pool ~ DVE ~ 70us; PE ~ 55us.

Tail (group-major loop so group 0's tail overlaps group 1's compute):
  dist1: combine group row-maxes, relu(-x), row-sum.
  dist2: Pool partition_all_reduce over runs[g] (max over the 128
  partition-interleaved rows), DMA-reshape one broadcast row [1,2048]
  to [128,16], relu(-x), row-sum.  Host sums the [128,4] partials.
"""

import os
import sys

import numpy as np

for _p in ("/opt/trn_rl_repo", "/root/.axon_site/_ro/trn_rl_repo"):
    if os.path.isdir(_p) and _p not in sys.path:
        sys.path.append(_p)

B, N, M, D = 8, 4096, 4096, 3
P = 128
NCORES = 8
GW = 2048          # m-group width = 4 PSUM banks of fp32
NG = M // GW       # 2 m-groups
NB = N // P        # 32 n-blocks
MMF = 512          # matmul moving free dim (one PSUM bank)
KA = 24            # augmented contraction rows (3-way bf16 split)
EVS = GW           # eviction column split: Act [0:EVS), DVE [EVS:GW)
J2S = 0            # unused (Pool has no generic tensor ops on HW)

_PROG = None


def _build_program(repeat: int = 1, variant: str = "full",
                   ev_mode: str = "cols", j2_mode: str = "cols",
                   act_frac: int = 21, j2s: int = J2S, evs: int = EVS):
    import concourse.mybir as mybir
    import concourse.tile as tile
    from concourse import bacc, bass_isa

    f32 = mybir.dt.float32
    bf16 = mybir.dt.bfloat16
    f16 = mybir.dt.float16
    u16 = mybir.dt.uint16
    Alu = mybir.AluOpType
    Ax = mybir.AxisListType

    nc = bacc.Bacc("TRN2", target_bir_lowering=False, debug=False,
                   num_devices=NCORES)
    a5d = nc.dram_tensor("a5", [KA, N], u16, kind="ExternalInput").ap()
    b5d = nc.dram_tensor("b5", [KA, M], u16, kind="ExternalInput").ap()
    outd = nc.dram_tensor("out", [P, 4], f32, kind="ExternalOutput").ap()

    with tile.TileContext(nc) as tc:
        with tc.tile_pool(name="persist", bufs=1) as persist:
            A5 = persist.tile([KA, N], bf16)
            B5 = persist.tile([KA, M], bf16)
            nc.sync.dma_start(A5[:].bitcast(u16), a5d[:])
            nc.sync.dma_start(B5[:].bitcast(u16), b5d[:])

            runs = [persist.tile([P, GW], f16, name=f"run{g}") for g in range(NG)]
            for g in range(NG):
                nc.vector.memset(runs[g][:], -60000.0)
            # per-tile row-max accumulators (dist1): col t = NB*g + i
            colmax = persist.tile([P, NB * NG], f32)

            outsb = persist.tile([P, 4], f32)
            nc.vector.memset(outsb[:], 0.0)

            # dist2 machinery: per-tile Pool partition_all_reduce results
            # are stacked (one DMA'd broadcast row per n-block) and merged
            # with the runs accumulator by one final PAR per group.
            stack = [persist.tile([NB + 1, GW], f16, name=f"stk{g}")
                     for g in range(NG)]
            for g in range(NG):
                nc.vector.memset(stack[g][:], -60000.0)
            fin = [persist.tile([NB + 1, GW], f16, name=f"fin{g}")
                   for g in range(NG)]
            parrun = [persist.tile([P, GW], f16, name=f"parrun{g}")
                      for g in range(NG)]
            st = [persist.tile([P, GW // P], f16, name=f"st{g}")
                  for g in range(NG)]

            from concourse.masks import make_identity
            ident = persist.tile([P, P], f32)
            make_identity(nc, ident[:])
            ident16 = persist.tile([P, P], f16)
            nc.vector.tensor_copy(ident16[:], ident[:])
            tmax = [persist.tile([P, GW // P], f32, name=f"tmax{g}")
                    for g in range(NG)]

            def tail_dist2(g, tp2):
                # transpose runs[g] through the PE; column-max per chunk
                for c in range(GW // P):
                    pt2 = tp2.tile([P, P], f16, tag="tr", name="pt2")
                    nc.tensor.transpose(pt2[:], runs[g][:, P * c:P * (c + 1)],
                                        ident16[:])
                    nc.vector.tensor_reduce(
                        tmax[g][:, c:c + 1], pt2[:], axis=Ax.X, op=Alu.max)
                neg = persist.tile([P, GW // P], f32, name=f"neg{g}")
                nc.vector.tensor_scalar(
                    out=neg[:], in0=tmax[g][:], scalar1=-1.0, scalar2=0.0,
                    op0=Alu.mult, op1=Alu.max)
                nc.vector.tensor_reduce(
                    outsb[:, 1 + g:2 + g], neg[:], axis=Ax.X, op=Alu.add)

            with tc.tile_pool(name="mpsum", bufs=2, space="PSUM") as mpsum, \
                 tc.tile_pool(name="evp", bufs=3) as evp, \
                 tc.tile_pool(name="parp", bufs=2) as parp, \
                 tc.tile_pool(name="scrp", bufs=1) as scrp:
                for rr in range(repeat):
                    for g in range(NG):
                        for i in range(NB):
                            t = NB * g + i
                            ps = mpsum.tile([P, GW], f32, tag="d2", name="ps")
                            for j in range(GW // MMF):
                                nc.tensor.matmul(
                                    ps[:, MMF * j:MMF * (j + 1)],
                                    lhsT=A5[:, P * i:P * (i + 1)],
                                    rhs=B5[:, GW * g + MMF * j:GW * g + MMF * (j + 1)],
                                    start=True, stop=True)
                            if variant == "pe":
                                continue
                            # interleave Act/Pool eviction: af of every 32
                            # tiles go to Act, spread evenly
                            # HW: only Act and DVE may read PSUM; Pool
                            # (GPSIMD) supports no generic tensor ops.
                            ev = evp.tile([P, GW], f16, tag="ev", name="ev")
                            if evs >= GW:
                                nc.scalar.copy(ev[:], ps[:])
                            else:
                                nc.scalar.copy(ev[:, 0:evs], ps[:, 0:evs])
                                nc.vector.tensor_copy(ev[:, evs:GW], ps[:, evs:GW])
                            if variant == "evict":
                                continue
                            if variant in ("full", "j1"):
                                scr = scrp.tile([P, GW], f16, tag="scr", name="scr")
                                nc.vector.tensor_scalar(
                                    out=scr[:], in0=ev[:], scalar1=-60000.0,
                                    scalar2=None, op0=Alu.max, op1=Alu.max,
                                    accum_out=colmax[:, t:t + 1])
                            if variant in ("full", "j2"):
                                # dist2: most tiles go through Pool PAR (the
                                # only fast Pool op on HW); the rest through
                                # the DVE runs accumulator.
                                if False and (t % 16) < 13:
                                    pr = parp.tile([P, GW], f16, tag="pr",
                                                   name="pr")
                                    nc.gpsimd.partition_all_reduce(
                                        pr[:], ev[:], channels=P,
                                        reduce_op=bass_isa.ReduceOp.max)
                                    nc.sync.dma_start(
                                        stack[g][i:i + 1, :], pr[0:1, :])
                                else:
                                    nc.vector.tensor_max(
                                        runs[g][:], ev[:], runs[g][:])


            if variant == "full":
                with tc.tile_pool(name="tpsum2", bufs=4, space="PSUM") as tp2:
                    for g in range(NG):
                        tail_dist2(g, tp2)
                # dist1: combine the two groups' per-tile maxes, relu(-x), sum
                d1 = persist.tile([P, NB], f32)
                nc.vector.tensor_max(d1[:], colmax[:, 0:NB], colmax[:, NB:2 * NB])
                d1r = persist.tile([P, NB], f32)
                nc.vector.tensor_scalar(
                    out=d1r[:], in0=d1[:], scalar1=-1.0, scalar2=0.0,
                    op0=Alu.mult, op1=Alu.max)
                nc.vector.tensor_reduce(outsb[:, 0:1], d1r[:], axis=Ax.X, op=Alu.add)

            nc.sync.dma_start(outd[:], outsb[:])

    nc.compile()
    return nc


def _get_program():
    global _PROG
    if _PROG is None:
        _PROG = _build_program()
    return _PROG


def _bf16_round(v: np.ndarray) -> np.ndarray:
    """Round f32 -> nearest-even bf16, returned as f32 with low bits zero."""
    u = np.ascontiguousarray(v, dtype=np.float32).view(np.uint32)
    r = ((u >> 16) & 1) + np.uint32(0x7FFF)
    return ((u + r) & np.uint32(0xFFFF0000)).view(np.float32)


# (i, j) split-product pairs kept: i + j <= 2
_PAIRS = [(0, 0), (0, 1), (0, 2), (1, 0), (1, 1), (2, 0)]


def _split3(v: np.ndarray):
    v = v.astype(np.float32)
    s0 = _bf16_round(v)
    s1 = _bf16_round(v - s0)
    s2 = _bf16_round(v - s0 - s1)
    return s0, s1, s2


def _prep_sides(x: np.ndarray, y: np.ndarray):
    """Build augmented [KA, N] / [KA, M] bf16 matrices (as uint16 bits).

    sum_k A[k,n] * Bm[k,m] = 2<x_n, y_m> - |x_n|^2 - |y_m|^2 = -d2[n,m]
    """
    xs = _split3(x)                      # x ~ xs0+xs1+xs2
    cs = _split3(2.0 * y)                # 2y ~ cs0+cs1+cs2
    nx = _split3((x.astype(np.float64) ** 2).sum(-1).astype(np.float32))
    nyn = _split3(-(y.astype(np.float64) ** 2).sum(-1).astype(np.float32))

    A = np.empty((KA, x.shape[0]), np.float32)
    Bm = np.empty((KA, y.shape[0]), np.float32)
    r = 0
    for d in range(D):
        for (ii, jj) in _PAIRS:
            A[r] = xs[ii][:, d]
            Bm[r] = cs[jj][:, d]
            r += 1
    for k in range(3):
        A[r] = nx[k]
        Bm[r] = -1.0
        r += 1
    for k in range(3):
        A[r] = 1.0
        Bm[r] = nyn[k]
        r += 1
    assert r == KA
    # values are bf16-rounded (-1, 1 exact): truncate to upper 16 bits
    Au = (np.ascontiguousarray(A).view(np.uint32) >> 16).astype(np.uint16)
    Bu = (np.ascontiguousarray(Bm).view(np.uint32) >> 16).astype(np.uint16)
    return Au, Bu


def kernel(xyz1: np.ndarray, xyz2: np.ndarray) -> np.ndarray:
    from concourse.bass_utils import run_bass_kernel_spmd

    xyz1 = np.asarray(xyz1, dtype=np.float32)
    xyz2 = np.asarray(xyz2, dtype=np.float32)
    assert xyz1.shape == (B, N, D) and xyz2.shape == (B, M, D)

    nc = _get_program()
    in_maps = []
    for b in range(NCORES):
        Au, Bu = _prep_sides(xyz1[b], xyz2[b])
        in_maps.append({"a5": Au, "b5": Bu})
    res = run_bass_kernel_spmd(nc, in_maps, list(range(NCORES))).results
    total = 0.0
    for r in res:
        o = r["out"].astype(np.float64)
        total += o[:, 0].sum() + o[:, 1].sum() + o[:, 2].sum()
    # mean(dist1) + mean(dist2) = (sum dist1 + sum dist2) / (B*N)   (N == M)
    return np.float32(total / (B * N))
